# revision 21
# baseline (speedup 1.0000x reference)
"""GraphSAGE 2-layer kernel for 8 Trainium2 NeuronCores.

Design: per-edge random-row gathers are descriptor-rate-bound on SWDGE.
This kernel runs FOUR SWDGE queues (ucode runs queue q's descriptor
generation on Q7 core pair 2q/2q+1, so queues generate concurrently),
taking the gather rate from ~8 ns/row to ~2.3 ns/row aggregate.  Every
gather is split into <=4 pieces round-robined across the queues so the
SDMA drain of the random reads is parallelized too and consumers can
start on early pieces.

  - Relabel nodes: degree-sorted serpentine deal into 392 blocks of 128 so
    every block has ~equal in-degree; 49 dst blocks per core.
  - UNIFIED table layout for both layers: x tables (layer 1) are host-
    reordered into the same (chunk, core, offset) layout that the layer-2
    AllGather produces, so ONE idx tensor and ONE dstT tensor drive both
    layers.  Sections = 3 AllGather chunks of 17/12/20 blocks (tables
    17408/12288/20480 rows, int16-indexable); chunk 1 is small so AG1
    completes before the gather queues need section-1 tables.
  - Layer 1 gathers RAW x rows (256B bf16) from the 3 replicated x tables;
    aggregation in transposed space: aggT[feat, slot] += g^T @ onehot on PE;
    h1T = relu(W1n^T (invd*aggT) + W1s^T xT) stays transposed.
  - Layer 2 pre-projects p2 = h1 @ W2n per block into [p2|0] 256B rows;
    THREE AllGathers (one per chunk) fire as their block ranges complete,
    overlapping the layer-1 gather stream; layer-2 gathers the cc tables
    with the SAME indices.  Layer-2 aggregation is TWO-PHASE: sections
    0+1 accumulate per block and dump a bf16 partial to SBUF (closing the
    PSUM chain so banks/pools rotate during AllGather 2); after AG2 a
    short section-2 chain finishes each block.  Output stored transposed,
    host fixes up.
  - One-hot dstT columns are numbered sections-0/1 first, then section 2,
    so each phase walks a contiguous chunk range (built on DVE with one
    chunk of lookahead so the PE never waits on is_equal).
  - One compiled SPMD program; all per-core variability lives in input
    tensors (gather indices, one-hot slot columns, invd, xT).
"""

import numpy as np
import ml_dtypes

N = 50000
E = 800000
IN_F, HID_F, OUT_F = 128, 64, 64
CORES = 8
P = 128
NB = 392           # total dst blocks
BPC = NB // CORES  # 49 blocks per core
R = BPC * P        # 6272 rows per core
NPAD = NB * P      # 50176
GRP = 7            # blocks per gather group
NGRP = (BPC + GRP - 1) // GRP
SCHUNK = 32        # one-hot columns per DVE is_equal op
SENT = 200.0       # sentinel slot (one-hot row becomes all-zero)
NQ = 4             # SWDGE queues
NPIECE = 4         # max pieces per gather (round-robined over queues)

S = 3                      # AllGather chunks == gather sections
CH_B = [17, 12, 20]        # blocks per chunk (small chunk 1 so AG1
CH_LO = [0, 17, 29]        # completes before queues need section 1)
CH_R = [b * P for b in CH_B]       # rows per core per chunk
CH_JLO = [lo * P for lo in CH_LO]  # first row of each chunk
_CH_R_A = np.array(CH_R, np.int64)
_CH_JLO_A = np.array(CH_JLO, np.int64)

_cache = {}


def _relabel(dst):
    deg = np.bincount(dst, minlength=N).astype(np.int64)
    inv_deg = (1.0 / np.maximum(deg, 1)).astype(np.float32)
    order = np.argsort(-deg, kind="stable").astype(np.int64)
    idx = np.arange(N, dtype=np.int64)
    rnd = idx // NB
    k = idx % NB
    b_of = np.where(rnd % 2 == 0, k, NB - 1 - k)
    blk = np.empty(N, np.int64)
    slot = np.empty(N, np.int64)
    blk[order] = b_of
    slot[order] = rnd
    pos = blk * P + slot          # old id -> new id
    old_of_new = np.full(NPAD, -1, np.int64)
    old_of_new[pos] = idx
    return pos, old_of_new, inv_deg


def _core_sections(nsrc_c, ndst_local):
    """Split one core's edges into (block, section) lists.

    Section = AllGather chunk of the src node; table index within the
    chunk's table (same layout for x tables and cc tables).
    Returns dict (b, s) -> (tbl_idx array, slot array)."""
    blk = (ndst_local >> 7).astype(np.int64)
    dslot = (ndst_local & 127).astype(np.float32)
    c_of = nsrc_c // R
    j = nsrc_c % R
    sec = np.searchsorted(_CH_JLO_A, j, side="right") - 1
    tbl = c_of * _CH_R_A[sec] + (j - _CH_JLO_A[sec])
    out = {}
    o = np.lexsort((tbl, sec, blk))
    blk_s, sec_s, tbl_s, slot_s = blk[o], sec[o], tbl[o], dslot[o]
    bounds = np.searchsorted(blk_s * S + sec_s, np.arange(BPC * S + 1))
    for b in range(BPC):
        for s in range(S):
            lo_i, hi_i = bounds[b * S + s], bounds[b * S + s + 1]
            out[(b, s)] = (tbl_s[lo_i:hi_i], slot_s[lo_i:hi_i])
    return out


def _prep(x, src, dst):
    pos, old_of_new, inv_deg = _relabel(dst)
    nsrc = pos[src.astype(np.int64)]
    ndst = pos[dst.astype(np.int64)]

    xp = np.zeros((NPAD, IN_F), np.float32)
    valid = old_of_new >= 0
    xp[valid] = x[old_of_new[valid]]
    invd_new = np.ones(NPAD, np.float32)
    invd_new[valid] = inv_deg[old_of_new[valid]]

    core_of_edge = ndst // R
    secs = []
    for c in range(CORES):
        m = core_of_edge == c
        secs.append(_core_sections(nsrc[m], ndst[m] - c * R))

    # Packed layout: per (g, s) gathers are the plain concat of the group's
    # blocks' edges (one pad-to-128 per group-section).
    Tgs = {}
    pref = [dict() for _ in range(CORES)]  # (b,s) -> rank offset
    for g in range(NGRP):
        blks = range(g * GRP, min((g + 1) * GRP, BPC))
        for s in range(S):
            tot = 0
            for c in range(CORES):
                acc = 0
                for b in blks:
                    pref[c][(b, s)] = acc
                    acc += len(secs[c][(b, s)][0])
                tot = max(tot, acc)
            Tgs[(g, s)] = (tot + P - 1) // P
    rng = {}
    for b in range(BPC):
        for s in range(S):
            t0, t1 = None, None
            for c in range(CORES):
                n = len(secs[c][(b, s)][0])
                if n == 0:
                    continue
                p0 = pref[c][(b, s)]
                a, z = p0 // P, (p0 + n - 1) // P
                t0 = a if t0 is None else min(t0, a)
                t1 = z if t1 is None else max(t1, z)
            rng[(b, s)] = (t0, t1) if t0 is not None else None
        if all(rng[(b, s)] is None for s in range(S)):
            rng[(b, 0)] = (0, 0)   # degenerate: one all-SENT job

    def build_core(sc, prefc):
        # gather idx per (group, section), packed; dstT columns: first all
        # section-0/1 jobs (b asc, s asc, t asc), then all section-2 jobs
        parts = []
        for g in range(NGRP):
            blks = range(g * GRP, min((g + 1) * GRP, BPC))
            for s in range(S):
                cat = np.concatenate(
                    [sc[(b, s)][0] for b in blks]) if Tgs[(g, s)] else \
                    np.zeros(0, np.int64)
                cap = Tgs[(g, s)] * P
                a = np.zeros(cap, np.int64)
                a[: len(cat)] = cat
                cw = cap // 16
                parts.append(np.tile(a.reshape(cw, 16).T.astype(np.int16),
                                     (8, 1)))
        idx_w = np.concatenate(parts, axis=1)

        def col_for(b, s):
            t0, t1 = rng[(b, s)]
            _, sl = sc[(b, s)]
            n = len(sl)
            p0 = prefc[(b, s)]
            cols = []
            for t in range(t0, t1 + 1):
                col = np.full(P, SENT, np.float32)
                r_lo = max(t * P, p0)
                r_hi = min((t + 1) * P, p0 + n)
                if r_hi > r_lo:
                    col[r_lo - t * P : r_hi - t * P] = \
                        sl[r_lo - p0 : r_hi - p0]
                cols.append(col)
            return cols

        cols = []
        for b in range(BPC):
            for s in (0, 1):
                if rng[(b, s)] is not None:
                    cols.extend(col_for(b, s))
        for b in range(BPC):
            if rng[(b, 2)] is not None:
                cols.extend(col_for(b, 2))
        dstT = np.stack(cols, axis=1).astype(ml_dtypes.bfloat16)
        return idx_w, dstT

    xp_bf = xp.astype(ml_dtypes.bfloat16)
    percore = []
    for c in range(CORES):
        iw, dT = build_core(secs[c], pref[c])
        xT = xp_bf[c * R : (c + 1) * R].T.copy()
        ivr = np.broadcast_to(invd_new[c * R : (c + 1) * R][None, :],
                              (P, R)).astype(ml_dtypes.bfloat16).copy()
        percore.append((iw, dT, xT, ivr))

    # x tables in (chunk, core, offset) layout == cc table layout
    xc = []
    for k in range(S):
        t = np.zeros((CORES * CH_R[k], IN_F), np.float32)
        for c in range(CORES):
            t[c * CH_R[k] : (c + 1) * CH_R[k]] = \
                xp[c * R + CH_JLO[k] : c * R + CH_JLO[k] + CH_R[k]]
        xc.append(t.astype(ml_dtypes.bfloat16))

    return pos, (Tgs, rng), xc, percore


def _build(L):
    import concourse.bacc as bacc
    import concourse.bass as bass  # noqa: F401
    import concourse.mybir as mybir
    import concourse.tile as tile

    f32 = mybir.dt.float32
    bf16 = mybir.dt.bfloat16
    i16 = mybir.dt.int16
    eq = mybir.AluOpType.is_equal
    mul = mybir.AluOpType.mult
    relu = mybir.ActivationFunctionType.Relu

    Tgs, rng = L
    # idx col offsets per (g, s)
    offs = {}
    w = 0
    for g in range(NGRP):
        for s in range(S):
            offs[(g, s)] = w
            w += Tgs[(g, s)] * P // 16
    # jobs01[b]: (g, s, t, col) for s in 0/1; jobs2[b]: for s == 2
    jobs01 = {b: [] for b in range(BPC)}
    jobs2 = {b: [] for b in range(BPC)}
    col = 0
    for b in range(BPC):
        g = b // GRP
        for s in (0, 1):
            if rng[(b, s)] is None:
                continue
            t0, t1 = rng[(b, s)]
            for t in range(t0, t1 + 1):
                jobs01[b].append((g, s, t, col))
                col += 1
    ncol01 = col
    for b in range(BPC):
        g = b // GRP
        if rng[(b, 2)] is None:
            continue
        t0, t1 = rng[(b, 2)]
        for t in range(t0, t1 + 1):
            jobs2[b].append((g, 2, t, col))
            col += 1
    ncol = col

    nc = bacc.Bacc("TRN2", target_bir_lowering=False, debug=False,
                   num_devices=CORES, num_swdge_queues=NQ,
                   dynamic_dma_scratch_size=38912)

    xc_d = [nc.dram_tensor(f"xc{k}", [CORES * CH_R[k], IN_F], bf16,
                           kind="ExternalInput") for k in range(S)]
    xT_d = nc.dram_tensor("xT", [P, R], bf16, kind="ExternalInput")
    ivr_d = nc.dram_tensor("ivr", [P, R], bf16, kind="ExternalInput")
    idx_d = nc.dram_tensor("idx", [P, w], i16, kind="ExternalInput")
    dst_d = nc.dram_tensor("dst", [P, ncol], bf16, kind="ExternalInput")
    iota_d = nc.dram_tensor("iota", [P, P], bf16, kind="ExternalInput")
    w1n_d = nc.dram_tensor("w1n", [IN_F, HID_F], bf16, kind="ExternalInput")
    w1s_d = nc.dram_tensor("w1s", [IN_F, HID_F], bf16, kind="ExternalInput")
    w2n_d = nc.dram_tensor("w2n", [HID_F, OUT_F], bf16, kind="ExternalInput")
    w2s_d = nc.dram_tensor("w2s", [HID_F, OUT_F], bf16, kind="ExternalInput")
    out_d = nc.dram_tensor("out", [OUT_F, R], f32, kind="ExternalOutput")

    cc_in = nc.dram_tensor("cc_in", [R, P], bf16)
    cc_c = [nc.dram_tensor(f"cc_c{k}", [CORES * CH_R[k], P], bf16)
            for k in range(S)]
    groups_all = [list(range(CORES))]

    qc = [0]  # round-robin queue counter over gather pieces

    with tile.TileContext(nc) as tc:
        with (
            tc.tile_pool(name="pers", bufs=1) as pers,
            tc.tile_pool(name="g0", bufs=2) as g0_pool,
            tc.tile_pool(name="g1", bufs=2) as g1_pool,
            tc.tile_pool(name="g2", bufs=3) as g2_pool,
            tc.tile_pool(name="sA", bufs=2) as spoolA,
            tc.tile_pool(name="sB", bufs=2) as spoolB,
            tc.tile_pool(name="stage", bufs=3) as stage,
            tc.tile_pool(name="pagg", bufs=2, space="PSUM") as pagg_pool,
            tc.tile_pool(name="pproj", bufs=2, space="PSUM") as pproj_pool,
            tc.tile_pool(name="pp2", bufs=2, space="PSUM") as pp2_pool,
        ):
            gpools = [g0_pool, g1_pool, g2_pool]

            def load(name, shape, dt, dram):
                t = pers.tile(shape, dt, tag=name)
                nc.sync.dma_start(out=t[:], in_=dram[:, :])
                return t

            idx = pers.tile([P, w], i16, tag="idx")
            w_head = offs[(1, 0)] if NGRP > 1 else w
            nc.sync.dma_start(out=idx[:, :w_head], in_=idx_d[:, :w_head])
            didx = pers.tile([P, 8], i16, tag="didx")
            nc.vector.memset(didx[:], 0)
            dout = pers.tile([P, P], bf16, tag="dout")
            nc.gpsimd.dma_gather(
                out_ap=dout[:].rearrange("p (t e) -> p t e", e=P),
                in_ap=xc_d[0][:, :], idxs_ap=didx[:],
                num_idxs=P, num_idxs_reg=P, elem_size=P,
                single_packet=False, queue_num=0)
            dst = load("dst", [P, ncol], bf16, dst_d)
            nc.sync.dma_start(out=idx[:, w_head:], in_=idx_d[:, w_head:])
            iota = load("iota", [P, P], bf16, iota_d)
            xT = load("xT", [P, R], bf16, xT_d)
            ivr = load("ivr", [P, R], bf16, ivr_d)
            w1n = load("w1n", [IN_F, HID_F], bf16, w1n_d)
            w1s = load("w1s", [IN_F, HID_F], bf16, w1s_d)
            w2n = load("w2n", [HID_F, OUT_F], bf16, w2n_d)
            w2s = load("w2s", [HID_F, OUT_F], bf16, w2s_d)
            h1T = pers.tile([HID_F, R], bf16)
            partT = pers.tile([HID_F, BPC * P], bf16)

            def gather_seq(tables, order, gtiles, npiece=NPIECE):
                """Emit gathers; each split into <=npiece pieces round-
                robined across the NQ queues (parallel gen + drain)."""
                for g, s in order:
                    ntile = Tgs[(g, s)]
                    if ntile == 0:
                        continue
                    gt = gpools[s].tile([P, ntile * P], bf16, tag=f"g{s}")
                    ns = min(npiece, ntile)
                    bnds = [ntile * i // ns for i in range(ns + 1)]
                    for a, bnd in zip(bnds[:-1], bnds[1:]):
                        nidx = (bnd - a) * P
                        q = qc[0] % NQ
                        qc[0] += 1
                        nc.gpsimd.dma_gather(
                            out_ap=gt[:, a * P : bnd * P].rearrange(
                                "p (t e) -> p t e", e=P),
                            in_ap=tables[s][:, :],
                            idxs_ap=idx[:, offs[(g, s)] + a * P // 16 :
                                         offs[(g, s)] + bnd * P // 16],
                            num_idxs=nidx, num_idxs_reg=nidx, elem_size=P,
                            single_packet=False, queue_num=q)
                    gtiles[(g, s)] = gt

            def make_schunks(pool, c_lo, c_hi):
                """One-hot chunk provider walking [c_lo, c_hi) monotonically
                with one chunk of build-ahead."""
                cache = {}

                def build(ci):
                    c0 = ci * SCHUNK
                    kk = min(SCHUNK, c_hi - c0)
                    st = pool.tile([P, SCHUNK * P], bf16, tag="s")
                    nc.vector.tensor_tensor(
                        out=st[:, : kk * P].rearrange(
                            "p (t q) -> p t q", t=kk),
                        in0=dst[:, c0 : c0 + kk].unsqueeze(2)
                            .to_broadcast([P, kk, P]),
                        in1=iota[:].unsqueeze(1).to_broadcast([P, kk, P]),
                        op=eq)
                    cache[ci] = st

                def s_for(c):
                    ci = c // SCHUNK
                    if ci not in cache:
                        build(ci)
                    if ci + 1 not in cache and (ci + 1) * SCHUNK < c_hi:
                        build(ci + 1)
                    return cache[ci], c % SCHUNK

                return s_for

            def chain(jl, gtiles, s_for, pg, close, open_=True):
                nj = len(jl)
                for i, (g_, s_, t_, c_) in enumerate(jl):
                    gt = gtiles[(g_, s_)]
                    st, cc = s_for(c_)
                    nc.tensor.matmul(
                        out=pg[:],
                        lhsT=gt[:, t_ * P : (t_ + 1) * P],
                        rhs=st[:, cc * P : (cc + 1) * P],
                        start=(open_ and i == 0),
                        stop=(close and i == nj - 1))

            def ag(k):
                nc.gpsimd.collective_compute(
                    "AllGather", mybir.AluOpType.bypass,
                    replica_groups=groups_all,
                    ins=[cc_in[CH_JLO[k] : CH_JLO[k] + CH_R[k], :].opt()],
                    outs=[cc_c[k].ap().opt()])

            # ---------------- layer 1 ----------------
            def emit1(b, pg):
                ts = stage.tile([P, P], bf16, tag="aggs")
                nc.vector.tensor_tensor(
                    out=ts[:], in0=pg[:],
                    in1=ivr[:, b * P : (b + 1) * P], op=mul)
                hp = pproj_pool.tile([HID_F, P], f32, tag="h1p")
                nc.tensor.matmul(out=hp[:], lhsT=w1n[:], rhs=ts[:],
                                 start=True, stop=False)
                nc.tensor.matmul(out=hp[:], lhsT=w1s[:],
                                 rhs=xT[:, b * P : (b + 1) * P],
                                 start=False, stop=True)
                nc.scalar.activation(out=h1T[:, b * P : (b + 1) * P],
                                     in_=hp[:], func=relu)
                p2 = pp2_pool.tile([P, HID_F], f32, tag="p2")
                nc.tensor.matmul(out=p2[:],
                                 lhsT=h1T[:, b * P : (b + 1) * P],
                                 rhs=w2n[:], start=True, stop=True)
                row = stage.tile([P, P], bf16, tag="row")
                nc.vector.memset(row[:, HID_F:], 0)
                nc.vector.tensor_copy(out=row[:, :HID_F], in_=p2[:])
                nc.sync.dma_start(out=cc_in[b * P : (b + 1) * P, :],
                                  in_=row[:])

            def run1(s_forA, s_forB, gtiles, b0, b1):
                for b in range(b0, b1):
                    pg = pagg_pool.tile([P, P], f32, tag="agg")
                    j01, j2 = jobs01[b], jobs2[b]
                    chain(j01, gtiles, s_forA, pg, close=not j2)
                    if j2:
                        chain(j2, gtiles, s_forB, pg, close=True,
                              open_=not j01)
                    emit1(b, pg)

            g1t = {}
            g2t = {}
            xtabs = tuple(xc_d)
            ctabs = tuple(cc_c)
            sf1A = make_schunks(spoolA, 0, ncol01)
            sf1B = make_schunks(spoolB, ncol01, ncol)

            gather_seq(xtabs, [(g, s) for g in range(NGRP)
                               for s in range(S)], g1t)
            run1(sf1A, sf1B, g1t, 0, CH_LO[1])
            ag(0)
            gather_seq(ctabs, [(g, 0) for g in range(3)], g2t)
            run1(sf1A, sf1B, g1t, CH_LO[1], CH_LO[2])
            ag(1)
            mid0 = [(g, 0) for g in range(3, NGRP)]
            mid1 = [(g, 1) for g in range(NGRP)]
            mid = []
            for i in range(max(len(mid0), len(mid1))):
                if i < len(mid0):
                    mid.append(mid0[i])
                if i < len(mid1):
                    mid.append(mid1[i])
            gather_seq(ctabs, mid, g2t)
            run1(sf1A, sf1B, g1t, CH_LO[2], BPC)
            ag(2)
            tail = [(g, 2) for g in range(NGRP)]
            gather_seq(ctabs, tail[:-2], g2t)
            gather_seq(ctabs, tail[-2:], g2t, npiece=8)

            # ---------------- layer 2 ----------------
            sf2A = make_schunks(spoolA, 0, ncol01)
            sf2B = make_schunks(spoolB, ncol01, ncol)

            # phase 1: sections 0+1 accumulate, dump bf16 partial
            for b in range(BPC):
                if not jobs01[b]:
                    nc.vector.memset(partT[:, b * P : (b + 1) * P], 0)
                    continue
                pg = pagg_pool.tile([P, P], f32, tag="agg")
                chain(jobs01[b], g2t, sf2A, pg, close=True)
                nc.scalar.copy(out=partT[:, b * P : (b + 1) * P],
                               in_=pg[:HID_F, :])

            # phase 2: section 2 + combine + output
            def emit2(b, pg):
                o2 = pproj_pool.tile([HID_F, P], f32, tag="o2")
                nc.tensor.matmul(out=o2[:], lhsT=w2s[:],
                                 rhs=h1T[:, b * P : (b + 1) * P],
                                 start=True, stop=True)
                if pg is not None:
                    sm = stage.tile([HID_F, P], f32, tag="sm")
                    nc.vector.tensor_add(out=sm[:], in0=pg[:HID_F, :],
                                         in1=partT[:, b * P : (b + 1) * P])
                else:
                    sm = partT[:, b * P : (b + 1) * P]
                nT = stage.tile([HID_F, P], f32, tag="nT")
                nc.vector.tensor_tensor(
                    out=nT[:], in0=sm[:],
                    in1=ivr[:HID_F, b * P : (b + 1) * P], op=mul)
                of = stage.tile([HID_F, P], f32, tag="of")
                nc.vector.tensor_add(out=of[:], in0=nT[:], in1=o2[:])
                oo = stage.tile([HID_F, P], f32, tag="oo")
                nc.scalar.activation(out=oo[:], in_=of[:], func=relu)
                nc.sync.dma_start(out=out_d[:, b * P : (b + 1) * P],
                                  in_=oo[:])

            for b in range(BPC):
                j2 = jobs2[b]
                if j2:
                    pg = pagg_pool.tile([P, P], f32, tag="agg")
                    chain(j2, g2t, sf2B, pg, close=True)
                else:
                    pg = None
                emit2(b, pg)

    nc.compile()
    return nc


def _run(inputs, trace=False, tmpdir=None):
    from concourse.bass_utils import run_bass_kernel_spmd

    x = np.asarray(inputs["x"], np.float32)
    src = np.asarray(inputs["src"])
    dst = np.asarray(inputs["dst"])
    b1 = np.asarray(inputs["b1"], np.float32)
    b2 = np.asarray(inputs["b2"], np.float32)
    assert not np.any(b1) and not np.any(b2), "nonzero bias unsupported"

    pos, L, xc, percore = _prep(x, src, dst)

    Tgs, rng = L
    key = (tuple(sorted(Tgs.items())),
           tuple(sorted((k, v) for k, v in rng.items())))
    if key not in _cache:
        _cache[key] = _build(L)
    nc = _cache[key]

    bf = ml_dtypes.bfloat16
    iota = np.broadcast_to(np.arange(P, dtype=np.float32),
                           (P, P)).astype(bf).copy()
    shared = {
        "iota": iota,
        "w1n": np.asarray(inputs["W1_neigh"], np.float32).astype(bf),
        "w1s": np.asarray(inputs["W1_self"], np.float32).astype(bf),
        "w2n": np.asarray(inputs["W2_neigh"], np.float32).astype(bf),
        "w2s": np.asarray(inputs["W2_self"], np.float32).astype(bf),
    }
    for k in range(S):
        shared[f"xc{k}"] = xc[k]
    in_maps = []
    for c in range(CORES):
        iw, dT, xT, ivr = percore[c]
        m = dict(shared)
        m.update({"idx": iw, "dst": dT, "xT": xT, "ivr": ivr})
        in_maps.append(m)

    res = run_bass_kernel_spmd(nc, in_maps, list(range(CORES)),
                               trace=trace, tmpdir=tmpdir)
    h2 = np.concatenate([res.results[c]["out"] for c in range(CORES)],
                        axis=1).T  # [NPAD, 64]
    out = h2[pos]
    return np.ascontiguousarray(out, dtype=np.float32), res


def kernel(**inputs) -> np.ndarray:
    out, _ = _run(inputs, trace=False)
    return out
